# revision 15
# baseline (speedup 1.0000x reference)
"""Trainium2 Bass kernel for ChannelDropout.

Computes out[b,c,t] = brain_sig[b,c,t] * kept[b,c] / (EPS + proba[b,c]) where
  kept[b,c]  = ||positions[b,c] - center|| > 0.2
  proba[b,c] = mean_j(||positions[b,c] - mc_centers[j]|| > 0.2)

Pure data parallel over the batch dim: 8 batches per core on 8 NeuronCores.
Per core the (8, 273, 3000) shard is flattened to (2184, 3000) rows; rows are
processed as 17 chunks of 128 (channels on SBUF partitions) plus an 8-row
tail. The per-row scale is computed once with tiny DVE ops (channels on
partitions, the 101 centers broadcast along the free dim), then the big tensor
is streamed through SBUF in 6 MB tiles and multiplied by the per-partition
scale. Memory-bound: ~52 MB of HBM traffic per core.
"""

import numpy as np
from contextlib import ExitStack

import concourse.bass as bass
import concourse.bacc as bacc
import concourse.tile as tile
from concourse import mybir
from concourse.bass_utils import run_bass_kernel_spmd

B, C, T = 64, 273, 3000
N_CORES = 8
BPC = B // N_CORES          # batches per core
RPC = BPC * C               # rows per core = 2184
P = 128
NFULL = RPC // P            # 17 full chunks
TAIL = RPC - NFULL * P      # 8 rows
NCH = NFULL + 1             # 18 chunks
NMC = 100                   # monte-carlo trials
EPS = 1e-8
G = 4                       # full chunks per streamed group (6 MB tiles)

# Threshold on squared distance that exactly reproduces
# sqrt_f32(d2) > f32(0.2) for every f32 d2 (sqrt_f32 is monotone, so compare
# d2 against the largest f32 whose correctly-rounded sqrt is <= f32(0.2)).
def _tau() -> float:
    c = np.float32(0.2)
    x = np.float32(c) * np.float32(c)
    # walk down to the boundary, then up
    while np.sqrt(x) > c:
        x = np.nextafter(x, np.float32(0.0), dtype=np.float32)
    while True:
        nxt = np.nextafter(x, np.float32(np.inf), dtype=np.float32)
        if np.sqrt(nxt) > c:
            break
        x = nxt
    return float(x)


TAU = _tau()

_NC_CACHE = None


def _build_nc(reps: int = 1, g: int = G, bufs: int = 3,
              store_engine: str = "sync") -> bass.Bass:
    f32 = mybir.dt.float32
    nc = bacc.Bacc(None, target_bir_lowering=False)
    x = nc.declare_dram_parameter("x", [RPC, T], f32, isOutput=False)
    # padded to NCH*P rows on the host (pad value far from all centers)
    pos = nc.declare_dram_parameter("pos", [NCH * P, 2], f32, isOutput=False)
    cen = nc.declare_dram_parameter("cen", [2 * (1 + NMC)], f32, isOutput=False)
    y = nc.declare_dram_parameter("y", [RPC, T], f32, isOutput=True)

    with tile.TileContext(nc) as tc, ExitStack() as ctx:
        singles = ctx.enter_context(tc.tile_pool(name="singles", bufs=1))
        main = ctx.enter_context(tc.tile_pool(name="main", bufs=bufs))
        for _ in range(reps):
            _emit_body(nc, tc, singles, main, x, pos, cen, y, g, store_engine)

    nc.finalize()
    return nc


def _emit_body(nc, tc, singles, main, x, pos, cen, y, g=G, store_engine="sync"):
    f32 = mybir.dt.float32
    if True:
        # --- per-row scale computation (tiny) ---
        # POS[p, 2k:2k+2] = positions row k*128+p; garbage lanes preset to a
        # far-away point so the tail chunk's unused partitions stay finite.
        POS = singles.tile([P, 2 * NCH], f32)
        # one DMA for all chunks: POS[p, 2k+j] = pos[128k+p, j]
        nc.sync.dma_start(
            out=POS.rearrange("p (k j) -> p k j", j=2),
            in_=pos[:, :].rearrange("(k p) j -> p k j", p=P),
        )
        # centers row broadcast to all partitions: [cx, cy, m0x, m0y, ...]
        CEN = singles.tile([P, 2 * (1 + NMC)], f32)
        nc.gpsimd.dma_start(out=CEN, in_=cen[None, :].broadcast_to([P, 2 * (1 + NMC)]))

        # all-pairs squared distances: (128 rows, NCH chunks, 101 centers)
        px = POS[:, 0::2][:, :, None].broadcast_to([P, NCH, 1 + NMC])
        py = POS[:, 1::2][:, :, None].broadcast_to([P, NCH, 1 + NMC])
        mx = CEN[:, 0::2][:, None, :].broadcast_to([P, NCH, 1 + NMC])
        my = CEN[:, 1::2][:, None, :].broadcast_to([P, NCH, 1 + NMC])

        dx = singles.tile([P, NCH, 1 + NMC], f32)
        dy = singles.tile([P, NCH, 1 + NMC], f32)
        nc.vector.tensor_tensor(out=dx, in0=px, in1=mx, op=mybir.AluOpType.subtract)
        nc.vector.tensor_tensor(out=dy, in0=py, in1=my, op=mybir.AluOpType.subtract)
        nc.vector.tensor_mul(dx, dx, dx)
        nc.vector.tensor_mul(dy, dy, dy)
        nc.vector.tensor_add(dx, dx, dy)  # dx now holds d2
        mask = singles.tile([P, NCH, 1 + NMC], f32)
        nc.vector.tensor_scalar(
            out=mask, in0=dx, scalar1=TAU, scalar2=None, op0=mybir.AluOpType.is_gt
        )
        # denom = (sum_mc / 100) + EPS ; scale = kept / denom
        denom = singles.tile([P, NCH], f32)
        nc.vector.tensor_reduce(
            out=denom, in_=mask[:, :, 1:], axis=mybir.AxisListType.X,
            op=mybir.AluOpType.add,
        )
        nc.vector.tensor_scalar(
            out=denom, in0=denom, scalar1=1.0 / NMC, scalar2=EPS,
            op0=mybir.AluOpType.mult, op1=mybir.AluOpType.add,
        )
        rden = singles.tile([P, NCH], f32)
        nc.vector.reciprocal(out=rden, in_=denom)
        scale = singles.tile([P, NCH, 1], f32)
        nc.vector.tensor_tensor(
            out=scale, in0=mask[:, :, 0:1], in1=rden[:, :, None],
            op=mybir.AluOpType.mult,
        )

        # --- streamed multiply of the big tensor ---
        store = getattr(nc, store_engine)
        starts = list(range(0, NFULL, g))
        for g0 in starts:
            k = min(g, NFULL - g0)
            r0 = g0 * P
            xt = main.tile([P, g, T], f32, tag="xt")
            src = x[r0 : r0 + k * P, :].rearrange("(k p) t -> p k t", p=P)
            nc.sync.dma_start(out=xt[:, :k, :], in_=src)
            nc.vector.tensor_tensor(
                out=xt[:, :k, :],
                in0=xt[:, :k, :],
                in1=scale[:, g0 : g0 + k, :].broadcast_to([P, k, T]),
                op=mybir.AluOpType.mult,
            )
            dst = y[r0 : r0 + k * P, :].rearrange("(k p) t -> p k t", p=P)
            store.dma_start(out=dst, in_=xt[:, :k, :])

        # tail rows (8, 3000)
        xt = main.tile([P, g, T], f32, tag="xt")
        r0 = NFULL * P
        nc.sync.dma_start(out=xt[:TAIL, 0, :], in_=x[r0:, :])
        nc.vector.tensor_tensor(
            out=xt[:TAIL, 0, :],
            in0=xt[:TAIL, 0, :],
            in1=scale[:TAIL, NFULL, :].broadcast_to([TAIL, T]),
            op=mybir.AluOpType.mult,
        )
        store.dma_start(out=y[r0:, :], in_=xt[:TAIL, 0, :])


def _get_nc() -> bass.Bass:
    global _NC_CACHE
    if _NC_CACHE is None:
        _NC_CACHE = _build_nc()
    return _NC_CACHE


def make_in_maps(brain_sig, positions, center, mc_centers):
    brain_sig = np.ascontiguousarray(brain_sig, dtype=np.float32)
    positions = np.ascontiguousarray(positions, dtype=np.float32)
    cen = np.concatenate(
        [np.asarray(center, np.float32).ravel(),
         np.asarray(mc_centers, np.float32).ravel()]
    )
    in_maps = []
    for i in range(N_CORES):
        p = np.full((NCH * P, 2), 10.0, dtype=np.float32)
        p[:RPC] = positions[i * BPC : (i + 1) * BPC].reshape(RPC, 2)
        in_maps.append({
            "x": brain_sig[i * BPC : (i + 1) * BPC].reshape(RPC, T),
            "pos": p,
            "cen": cen,
        })
    return in_maps


def kernel(brain_sig, positions, center, mc_centers):
    nc = _get_nc()
    in_maps = make_in_maps(brain_sig, positions, center, mc_centers)
    res = run_bass_kernel_spmd(nc, in_maps, list(range(N_CORES)))
    out = np.concatenate(
        [r["y"].reshape(BPC, C, T) for r in res.results], axis=0
    )
    return out.astype(np.float32, copy=False)


# revision 19
# speedup vs baseline: 1.3744x; 1.3744x over previous
"""Trainium2 Bass kernel for ChannelDropout.

Computes out[b,c,t] = brain_sig[b,c,t] * kept[b,c] / (EPS + proba[b,c]) where
  kept[b,c]  = ||positions[b,c] - center|| > 0.2
  proba[b,c] = mean_j(||positions[b,c] - mc_centers[j]|| > 0.2)

Pure data parallel over the batch dim: 8 batches per core on 8 NeuronCores.
Per core the (8, 273, 3000) shard is flattened to (2184, 3000) rows; rows are
processed as 17 chunks of 128 (channels on SBUF partitions) plus an 8-row
tail. The per-row scale is computed once with tiny DVE ops (channels on
partitions, the 101 centers broadcast along the free dim), then the big tensor
is streamed through SBUF in 6 MB tiles and multiplied by the per-partition
scale. Memory-bound: ~52 MB of HBM traffic per core.
"""

import numpy as np
from contextlib import ExitStack

import concourse.bass as bass
import concourse.bacc as bacc
import concourse.tile as tile
from concourse import mybir
from concourse.bass_utils import run_bass_kernel_spmd

B, C, T = 64, 273, 3000
N_CORES = 8
BPC = B // N_CORES          # batches per core
RPC = BPC * C               # rows per core = 2184
P = 128
NFULL = RPC // P            # 17 full chunks
TAIL = RPC - NFULL * P      # 8 rows
NCH = NFULL + 1             # 18 chunks
NMC = 100                   # monte-carlo trials
EPS = 1e-8
G = 4                       # full chunks per streamed group (6 MB tiles)
BUFS = 3                    # streamed-tile buffers (load/mult/store overlap)

# Threshold on squared distance that exactly reproduces
# sqrt_f32(d2) > f32(0.2) for every f32 d2 (sqrt_f32 is monotone, so compare
# d2 against the largest f32 whose correctly-rounded sqrt is <= f32(0.2)).
def _tau() -> float:
    c = np.float32(0.2)
    x = np.float32(c) * np.float32(c)
    # walk down to the boundary, then up
    while np.sqrt(x) > c:
        x = np.nextafter(x, np.float32(0.0), dtype=np.float32)
    while True:
        nxt = np.nextafter(x, np.float32(np.inf), dtype=np.float32)
        if np.sqrt(nxt) > c:
            break
        x = nxt
    return float(x)


TAU = _tau()

_NC_CACHE = None


def _build_nc(reps: int = 1, g: int = G, bufs: int = BUFS,
              store_engine: str = "sync") -> bass.Bass:
    f32 = mybir.dt.float32
    nc = bacc.Bacc(None, target_bir_lowering=False)
    x = nc.declare_dram_parameter("x", [RPC, T], f32, isOutput=False)
    # padded to NCH*P rows on the host (pad value far from all centers)
    pos = nc.declare_dram_parameter("pos", [NCH * P, 2], f32, isOutput=False)
    cen = nc.declare_dram_parameter("cen", [2 * (1 + NMC)], f32, isOutput=False)
    y = nc.declare_dram_parameter("y", [RPC, T], f32, isOutput=True)

    with tile.TileContext(nc) as tc, ExitStack() as ctx:
        singles = ctx.enter_context(tc.tile_pool(name="singles", bufs=1))
        main = ctx.enter_context(tc.tile_pool(name="main", bufs=bufs))
        for _ in range(reps):
            _emit_body(nc, tc, singles, main, x, pos, cen, y, g, store_engine)

    nc.finalize()
    return nc


def _emit_body(nc, tc, singles, main, x, pos, cen, y, g=G, store_engine="sync"):
    f32 = mybir.dt.float32
    store = getattr(nc, store_engine)

    # POS[p, 2k:2k+2] = positions row k*128+p (host-padded to NCH*P rows)
    POS = singles.tile([P, 2 * NCH], f32)
    nc.sync.dma_start(
        out=POS.rearrange("p (k j) -> p k j", j=2),
        in_=pos[:, :].rearrange("(k p) j -> p k j", p=P),
    )
    # centers row broadcast to all partitions: [cx, cy, m0x, m0y, ...]
    CEN = singles.tile([P, 2 * (1 + NMC)], f32)
    nc.gpsimd.dma_start(out=CEN, in_=cen[None, :].broadcast_to([P, 2 * (1 + NMC)]))

    def emit_scale(k0, k1, tag):
        """Per-row scale for chunks [k0, k1): kept / (EPS + mean_mc)."""
        n = k1 - k0
        px = POS[:, 2 * k0 :: 2][:, :n, None].broadcast_to([P, n, 1 + NMC])
        py = POS[:, 2 * k0 + 1 :: 2][:, :n, None].broadcast_to([P, n, 1 + NMC])
        mx = CEN[:, 0::2][:, None, :].broadcast_to([P, n, 1 + NMC])
        my = CEN[:, 1::2][:, None, :].broadcast_to([P, n, 1 + NMC])
        dx = singles.tile([P, n, 1 + NMC], f32, tag=f"dx{tag}")
        dy = singles.tile([P, n, 1 + NMC], f32, tag=f"dy{tag}")
        nc.vector.tensor_tensor(out=dx, in0=px, in1=mx, op=mybir.AluOpType.subtract)
        nc.vector.tensor_tensor(out=dy, in0=py, in1=my, op=mybir.AluOpType.subtract)
        nc.vector.tensor_mul(dx, dx, dx)
        nc.vector.tensor_mul(dy, dy, dy)
        nc.vector.tensor_add(dx, dx, dy)  # dx now holds d2
        nc.vector.tensor_scalar(
            out=dx, in0=dx, scalar1=TAU, scalar2=None, op0=mybir.AluOpType.is_gt
        )
        denom = singles.tile([P, n], f32, tag=f"den{tag}")
        nc.vector.tensor_reduce(
            out=denom, in_=dx[:, :, 1:], axis=mybir.AxisListType.X,
            op=mybir.AluOpType.add,
        )
        nc.vector.tensor_scalar(
            out=denom, in0=denom, scalar1=1.0 / NMC, scalar2=EPS,
            op0=mybir.AluOpType.mult, op1=mybir.AluOpType.add,
        )
        nc.vector.reciprocal(out=denom, in_=denom)
        sc = singles.tile([P, n, 1], f32, tag=f"sc{tag}")
        nc.vector.tensor_tensor(
            out=sc, in0=dx[:, :, 0:1], in1=denom[:, :, None],
            op=mybir.AluOpType.mult,
        )
        return sc

    def emit_group(g0, k, sc, sck0):
        """Load chunks [g0, g0+k), multiply by scale columns, store."""
        r0 = g0 * P
        xt = main.tile([P, g, T], f32, tag="xt")
        src = x[r0 : r0 + k * P, :].rearrange("(k p) t -> p k t", p=P)
        nc.sync.dma_start(out=xt[:, :k, :], in_=src)
        nc.vector.tensor_tensor(
            out=xt[:, :k, :],
            in0=xt[:, :k, :],
            in1=sc[:, g0 - sck0 : g0 - sck0 + k, :].broadcast_to([P, k, T]),
            op=mybir.AluOpType.mult,
        )
        dst = y[r0 : r0 + k * P, :].rearrange("(k p) t -> p k t", p=P)
        store.dma_start(out=dst, in_=xt[:, :k, :])

    # scale for the first group + the ragged tail, then their data ops, then
    # the remaining scales, then the remaining groups.  This keeps the first
    # multiplies off the critical path of the full scale pipeline and buries
    # the inefficient 8-row tail transfer under the main stream.
    k_a = min(g, NFULL)
    sc_a = emit_scale(0, k_a, "a")
    sc_t = emit_scale(NFULL, NCH, "t")

    # ragged tail rows (TAIL, T), processed early
    r0 = NFULL * P
    xt = main.tile([P, g, T], f32, tag="xt")
    nc.sync.dma_start(out=xt[:TAIL, 0, :], in_=x[r0:, :])
    nc.vector.tensor_tensor(
        out=xt[:TAIL, 0, :],
        in0=xt[:TAIL, 0, :],
        in1=sc_t[:TAIL, 0, :].broadcast_to([TAIL, T]),
        op=mybir.AluOpType.mult,
    )
    store.dma_start(out=y[r0:, :], in_=xt[:TAIL, 0, :])

    emit_group(0, k_a, sc_a, 0)
    sc_b = emit_scale(k_a, NFULL, "b") if k_a < NFULL else None
    for g0 in range(k_a, NFULL, g):
        k = min(g, NFULL - g0)
        emit_group(g0, k, sc_b, k_a)


def _get_nc() -> bass.Bass:
    global _NC_CACHE
    if _NC_CACHE is None:
        _NC_CACHE = _build_nc()
    return _NC_CACHE


def make_in_maps(brain_sig, positions, center, mc_centers):
    brain_sig = np.ascontiguousarray(brain_sig, dtype=np.float32)
    positions = np.ascontiguousarray(positions, dtype=np.float32)
    cen = np.concatenate(
        [np.asarray(center, np.float32).ravel(),
         np.asarray(mc_centers, np.float32).ravel()]
    )
    in_maps = []
    for i in range(N_CORES):
        p = np.full((NCH * P, 2), 10.0, dtype=np.float32)
        p[:RPC] = positions[i * BPC : (i + 1) * BPC].reshape(RPC, 2)
        in_maps.append({
            "x": brain_sig[i * BPC : (i + 1) * BPC].reshape(RPC, T),
            "pos": p,
            "cen": cen,
        })
    return in_maps


def kernel(brain_sig, positions, center, mc_centers):
    nc = _get_nc()
    in_maps = make_in_maps(brain_sig, positions, center, mc_centers)
    res = run_bass_kernel_spmd(nc, in_maps, list(range(N_CORES)))
    out = np.concatenate(
        [r["y"].reshape(BPC, C, T) for r in res.results], axis=0
    )
    return out.astype(np.float32, copy=False)


# revision 22
# speedup vs baseline: 1.4847x; 1.0802x over previous
"""Trainium2 Bass kernel for ChannelDropout.

Computes out[b,c,t] = brain_sig[b,c,t] * kept[b,c] / (EPS + proba[b,c]) where
  kept[b,c]  = ||positions[b,c] - center|| > 0.2
  proba[b,c] = mean_j(||positions[b,c] - mc_centers[j]|| > 0.2)

Pure data parallel over the batch dim: 8 batches per core on 8 NeuronCores.
Per core the (8, 273, 3000) shard is flattened to (2184, 3000) rows; rows are
processed as 17 chunks of 128 (channels on SBUF partitions) plus an 8-row
tail. The per-row scale is computed once with tiny DVE ops (channels on
partitions, the 101 centers broadcast along the free dim), then the big tensor
is streamed through SBUF in 6 MB tiles and multiplied by the per-partition
scale. Memory-bound: ~52 MB of HBM traffic per core.
"""

import numpy as np
from contextlib import ExitStack

import concourse.bass as bass
import concourse.bacc as bacc
import concourse.tile as tile
from concourse import mybir
from concourse.bass_utils import run_bass_kernel_spmd

B, C, T = 64, 273, 3000
N_CORES = 8
BPC = B // N_CORES          # batches per core
RPC = BPC * C               # rows per core = 2184
P = 128
NFULL = RPC // P            # 17 full chunks
TAIL = RPC - NFULL * P      # 8 rows
NCH = NFULL + 1             # 18 chunks
NMC = 100                   # monte-carlo trials
EPS = 1e-8
G = 4                       # max full chunks per streamed group (6 MB tiles)
BUFS = 3                    # streamed-tile buffers (load/mult/store overlap)
# Group size schedule (sums to NFULL=17).  Loads saturate the DMA engines
# from t=0 regardless of group size, so big groups win on transfer
# efficiency everywhere except the END: the final store serializes after the
# final multiply, so the last group is kept small (1.5 MB drain tail).
GROUPS = (4, 4, 4, 4, 1)

# Threshold on squared distance that exactly reproduces
# sqrt_f32(d2) > f32(0.2) for every f32 d2 (sqrt_f32 is monotone, so compare
# d2 against the largest f32 whose correctly-rounded sqrt is <= f32(0.2)).
def _tau() -> float:
    c = np.float32(0.2)
    x = np.float32(c) * np.float32(c)
    # walk down to the boundary, then up
    while np.sqrt(x) > c:
        x = np.nextafter(x, np.float32(0.0), dtype=np.float32)
    while True:
        nxt = np.nextafter(x, np.float32(np.inf), dtype=np.float32)
        if np.sqrt(nxt) > c:
            break
        x = nxt
    return float(x)


TAU = _tau()

_NC_CACHE = None


def _build_nc(reps: int = 1, g: int = G, bufs: int = BUFS,
              store_engine: str = "sync") -> bass.Bass:
    f32 = mybir.dt.float32
    nc = bacc.Bacc(None, target_bir_lowering=False)
    x = nc.declare_dram_parameter("x", [RPC, T], f32, isOutput=False)
    # padded to NCH*P rows on the host (pad value far from all centers)
    pos = nc.declare_dram_parameter("pos", [NCH * P, 2], f32, isOutput=False)
    cen = nc.declare_dram_parameter("cen", [2 * (1 + NMC)], f32, isOutput=False)
    y = nc.declare_dram_parameter("y", [RPC, T], f32, isOutput=True)

    with tile.TileContext(nc) as tc, ExitStack() as ctx:
        singles = ctx.enter_context(tc.tile_pool(name="singles", bufs=1))
        main = ctx.enter_context(tc.tile_pool(name="main", bufs=bufs))
        for _ in range(reps):
            _emit_body(nc, tc, singles, main, x, pos, cen, y, g, store_engine)

    nc.finalize()
    return nc


def _emit_body(nc, tc, singles, main, x, pos, cen, y, g=G, store_engine="sync"):
    f32 = mybir.dt.float32
    store = getattr(nc, store_engine)

    # POS[p, 2k:2k+2] = positions row k*128+p (host-padded to NCH*P rows)
    POS = singles.tile([P, 2 * NCH], f32)
    nc.sync.dma_start(
        out=POS.rearrange("p (k j) -> p k j", j=2),
        in_=pos[:, :].rearrange("(k p) j -> p k j", p=P),
    )
    # centers row broadcast to all partitions: [cx, cy, m0x, m0y, ...]
    CEN = singles.tile([P, 2 * (1 + NMC)], f32)
    nc.gpsimd.dma_start(out=CEN, in_=cen[None, :].broadcast_to([P, 2 * (1 + NMC)]))

    def emit_scale(k0, k1, tag):
        """Per-row scale for chunks [k0, k1): kept / (EPS + mean_mc)."""
        n = k1 - k0
        px = POS[:, 2 * k0 :: 2][:, :n, None].broadcast_to([P, n, 1 + NMC])
        py = POS[:, 2 * k0 + 1 :: 2][:, :n, None].broadcast_to([P, n, 1 + NMC])
        mx = CEN[:, 0::2][:, None, :].broadcast_to([P, n, 1 + NMC])
        my = CEN[:, 1::2][:, None, :].broadcast_to([P, n, 1 + NMC])
        dx = singles.tile([P, n, 1 + NMC], f32, tag=f"dx{tag}")
        dy = singles.tile([P, n, 1 + NMC], f32, tag=f"dy{tag}")
        nc.vector.tensor_tensor(out=dx, in0=px, in1=mx, op=mybir.AluOpType.subtract)
        nc.vector.tensor_tensor(out=dy, in0=py, in1=my, op=mybir.AluOpType.subtract)
        nc.vector.tensor_mul(dx, dx, dx)
        nc.vector.tensor_mul(dy, dy, dy)
        nc.vector.tensor_add(dx, dx, dy)  # dx now holds d2
        nc.vector.tensor_scalar(
            out=dx, in0=dx, scalar1=TAU, scalar2=None, op0=mybir.AluOpType.is_gt
        )
        denom = singles.tile([P, n], f32, tag=f"den{tag}")
        nc.vector.tensor_reduce(
            out=denom, in_=dx[:, :, 1:], axis=mybir.AxisListType.X,
            op=mybir.AluOpType.add,
        )
        nc.vector.tensor_scalar(
            out=denom, in0=denom, scalar1=1.0 / NMC, scalar2=EPS,
            op0=mybir.AluOpType.mult, op1=mybir.AluOpType.add,
        )
        nc.vector.reciprocal(out=denom, in_=denom)
        sc = singles.tile([P, n, 1], f32, tag=f"sc{tag}")
        nc.vector.tensor_tensor(
            out=sc, in0=dx[:, :, 0:1], in1=denom[:, :, None],
            op=mybir.AluOpType.mult,
        )
        return sc

    def emit_group(g0, k, sc, sck0):
        """Load chunks [g0, g0+k), multiply by scale columns, store."""
        r0 = g0 * P
        xt = main.tile([P, g, T], f32, tag="xt")
        src = x[r0 : r0 + k * P, :].rearrange("(k p) t -> p k t", p=P)
        nc.sync.dma_start(out=xt[:, :k, :], in_=src)
        nc.vector.tensor_tensor(
            out=xt[:, :k, :],
            in0=xt[:, :k, :],
            in1=sc[:, g0 - sck0 : g0 - sck0 + k, :].broadcast_to([P, k, T]),
            op=mybir.AluOpType.mult,
        )
        dst = y[r0 : r0 + k * P, :].rearrange("(k p) t -> p k t", p=P)
        store.dma_start(out=dst, in_=xt[:, :k, :])

    # Group schedule: sizes from GROUPS when g == G, else uniform g.
    if g == G:
        sizes = list(GROUPS)
        assert sum(sizes) == NFULL
    else:
        sizes = [min(g, NFULL - s) for s in range(0, NFULL, g)]
    starts = [sum(sizes[:i]) for i in range(len(sizes))]

    # scale for the first two groups + the ragged tail, then their data ops,
    # then the remaining scales, then the remaining groups.  This keeps the
    # first multiplies off the critical path of the full scale pipeline and
    # buries the inefficient 8-row tail transfer under the main stream.
    k_a = sizes[0] + (sizes[1] if len(sizes) > 1 else 0)
    sc_a = emit_scale(0, k_a, "a")
    sc_t = emit_scale(NFULL, NCH, "t")

    emit_group(starts[0], sizes[0], sc_a, 0)

    # ragged tail rows (TAIL, T), processed early
    r0 = NFULL * P
    xt = main.tile([P, g, T], f32, tag="xt")
    nc.sync.dma_start(out=xt[:TAIL, 0, :], in_=x[r0:, :])
    nc.vector.tensor_tensor(
        out=xt[:TAIL, 0, :],
        in0=xt[:TAIL, 0, :],
        in1=sc_t[:TAIL, 0, :].broadcast_to([TAIL, T]),
        op=mybir.AluOpType.mult,
    )
    store.dma_start(out=y[r0:, :], in_=xt[:TAIL, 0, :])

    if len(sizes) > 1:
        emit_group(starts[1], sizes[1], sc_a, 0)
    sc_b = emit_scale(k_a, NFULL, "b") if k_a < NFULL else None
    for g0, k in zip(starts[2:], sizes[2:]):
        emit_group(g0, k, sc_b, k_a)


def _get_nc() -> bass.Bass:
    global _NC_CACHE
    if _NC_CACHE is None:
        _NC_CACHE = _build_nc()
    return _NC_CACHE


def make_in_maps(brain_sig, positions, center, mc_centers):
    brain_sig = np.ascontiguousarray(brain_sig, dtype=np.float32)
    positions = np.ascontiguousarray(positions, dtype=np.float32)
    cen = np.concatenate(
        [np.asarray(center, np.float32).ravel(),
         np.asarray(mc_centers, np.float32).ravel()]
    )
    in_maps = []
    for i in range(N_CORES):
        p = np.full((NCH * P, 2), 10.0, dtype=np.float32)
        p[:RPC] = positions[i * BPC : (i + 1) * BPC].reshape(RPC, 2)
        in_maps.append({
            "x": brain_sig[i * BPC : (i + 1) * BPC].reshape(RPC, T),
            "pos": p,
            "cen": cen,
        })
    return in_maps


def kernel(brain_sig, positions, center, mc_centers):
    nc = _get_nc()
    in_maps = make_in_maps(brain_sig, positions, center, mc_centers)
    res = run_bass_kernel_spmd(nc, in_maps, list(range(N_CORES)))
    out = np.concatenate(
        [r["y"].reshape(BPC, C, T) for r in res.results], axis=0
    )
    return out.astype(np.float32, copy=False)
